# revision 28
# baseline (speedup 1.0000x reference)
"""Trainium2 Bass kernel for nn_BiLinAntisymmetricFunc.

Math: out[b,n] = g(x1[b,n]) - g(x2[b,n]) + sum_k alpha_k * x1^T (U_k V_k^T - V_k U_k^T) x2

The bilinear part collapses: with S = sum_k alpha_k (U_k V_k^T - V_k U_k^T)
(a precomputable [D,D] weight matrix), bili = x1^T S x2 per row =
rowsum(x1 . (x2 @ (-S))). That is ONE [N,D]@[D,D] matmul instead of four
[N,D]@[D,512] projections (2x fewer FLOPs).

Sharding: data-parallel over the 65536 rows (B*NR) -> 8 cores x 8192 rows.

All matmul operands are bf16 (host-cast): PE runs at the same rate as f32r,
but bf16 halves HBM traffic. Per core we stream x1 (row-major, for the
fused dot), x1^T and x2^T (d-major, for the matmuls) packed into ONE 3MB
DMA per 512-row pair, alternating between the two HWDGE rings (SP/ACT) so
per-transfer fixed costs don't serialize on a single FIFO.

Per 512-row pair, on device:
  - h1^T = relu(W1^T x^T + b1) for x1/x2, 4-way column-tiled
    (tile_position=(0,32j), n-slice j per strip); the MLP chain stays
    strip-local (W2/W3/biases replicated per 32-partition strip).
  - h2cat = relu(W2^T h1 + b2) for both tensors packed into 16 partition
    rows via zero-padded W2 stationaries (x1 -> rows 0:8, x2 -> rows 8:16).
  - gdiff = (h2cat strip)^T @ [W3; -W3] -> [128,1] per row-tile.
  - P = x2 @ (-S): lhsT = x2^T chunks (stationary), rhs = S chunks, f32 PSUM.
  - bili via DVE scalar_tensor_tensor: out=(x1 . P), accum_out=rowsum;
    fin = acc0 + acc1 + gdiff written into a [128,4] staging tile; one
    output DMA per pair (out layout [npairs,128,4], unpermuted on host).
"""

import os

import numpy as np

D, K, RANK = 1024, 8, 64
B, NR = 16, 4096
NCORES = 8
TOTAL_ROWS = B * NR
ROWS = TOTAL_ROWS // NCORES  # 8192 rows per core

MM_DT = os.environ.get("BILIN_MM_DT", "bf16")
MLP_MODE = os.environ.get("BILIN_MLP", "strips")  # "strips" | "plain"

_PROG_CACHE = {}


def _build_program(rows, mm_dt, variant=None, reps=1):
    variant = variant if variant is not None else os.environ.get("BILIN_VARIANT", "")
    mlp_mode = MLP_MODE
    if "plain" in variant:
        mlp_mode = "plain"
    if "strips" in variant:
        mlp_mode = "strips"
    if "strips2" in variant:
        mlp_mode = "strips2"
    nomlp = "nomlp" in variant
    dmaonly = "dmaonly" in variant
    computeonly = "computeonly" in variant
    nodot = "nodot" in variant
    empty = "empty" in variant
    if dmaonly or nodot:
        nomlp = True
    import concourse.bacc as bacc
    import concourse.bass as bass
    import concourse.mybir as mybir
    import concourse.tile as tile

    f32 = mybir.dt.float32
    bf16 = mybir.dt.bfloat16
    if mm_dt == "bf16":
        mdt = bf16
    elif mm_dt == "f32r":
        mdt = mybir.dt.float32r
    else:
        mdt = f32
    hdt = bf16  # tiny MLP h-chain dtype (negligible output contribution)

    nc = bacc.Bacc("TRN2", target_bir_lowering=False, debug=False)

    NCHUNK = D // 128  # 8 contraction chunks
    PAIR = 1024
    npairs = rows // PAIR
    XW = 3 * (PAIR // 128) * D  # packed row: [x1 | x1t | x2t], 4K cols each

    xx_d = nc.dram_tensor("xx", [npairs, 128, XW], mdt, kind="ExternalInput")
    s_d = nc.dram_tensor("s", [128, NCHUNK * D], mdt, kind="ExternalInput")  # packed -S
    w1_d = nc.dram_tensor("w1", [128, NCHUNK * K], mdt, kind="ExternalInput")
    b1_d = nc.dram_tensor("b1", [128, 1], f32, kind="ExternalInput")
    b2_d = nc.dram_tensor("b2", [128, 1], f32, kind="ExternalInput")
    w2a_d = nc.dram_tensor("w2a", [128, 2 * K], hdt, kind="ExternalInput")  # [W2|0]
    w2b_d = nc.dram_tensor("w2b", [128, 2 * K], hdt, kind="ExternalInput")  # [0|W2]
    w3_d = nc.dram_tensor("w3", [128, 1], hdt, kind="ExternalInput")  # [W3;-W3] per strip
    out_d = nc.dram_tensor("out", [npairs, 128, PAIR // 128], f32, kind="ExternalOutput")

    relu = mybir.ActivationFunctionType.Relu
    mult = mybir.AluOpType.mult

    with tile.TileContext(nc) as tc:
        with (
            tc.tile_pool(name="const", bufs=1) as cpool,
            tc.tile_pool(name="xx", bufs=2) as xxpool,
            tc.tile_pool(name="hsb", bufs=6) as hpool,
            tc.tile_pool(name="scr", bufs=2) as scrpool,
            tc.tile_pool(name="acc", bufs=40) as accpool,
            tc.tile_pool(name="st", bufs=3) as stpool,
            tc.tile_pool(name="pp", bufs=4, space="PSUM") as ppool,
            tc.tile_pool(name="hp", bufs=3, space="PSUM") as hppool,
            tc.tile_pool(name="gp", bufs=1, space="PSUM") as gppool,
        ):
            # ---- resident constants ----
            s_sb = cpool.tile([128, NCHUNK * D], mdt)
            nc.sync.dma_start(s_sb[:], s_d[:, :])
            w1_sb = cpool.tile([128, NCHUNK * K], mdt)
            nc.sync.dma_start(w1_sb[:], w1_d[:, :])
            w2a_sb = cpool.tile([128, 2 * K], hdt)
            nc.sync.dma_start(w2a_sb[:], w2a_d[:, :])
            w2b_sb = cpool.tile([128, 2 * K], hdt)
            nc.sync.dma_start(w2b_sb[:], w2b_d[:, :])
            w3_sb = cpool.tile([128, 1], hdt)
            nc.sync.dma_start(w3_sb[:], w3_d[:, :])
            b1_sb = cpool.tile([128, 1], f32)
            nc.sync.dma_start(b1_sb[:], b1_d[:, :])
            b2_sb = cpool.tile([128, 1], f32)
            nc.sync.dma_start(b2_sb[:], b2_d[:, :])

            def emit_mlp_plain(p, x1t_t, x2t_t):
                """h chains with k on partitions [0:16); h2cat [16, PAIR] sbuf."""
                h1sb = []
                for j, xt_t in enumerate((x1t_t, x2t_t)):
                    h1ps = hppool.tile([128, PAIR], f32, tag="hps", name=f"h1ps{p}_{j}")
                    for c in range(NCHUNK):
                        nc.tensor.matmul(
                            h1ps[0:K, :],
                            w1_sb[:, c * K : (c + 1) * K],
                            xt_t[:, c * PAIR : (c + 1) * PAIR],
                            start=(c == 0),
                            stop=(c == NCHUNK - 1),
                        )
                    h1s = hpool.tile([K, PAIR], hdt, tag="h1sb", name=f"h1sb{p}_{j}")
                    nc.scalar.activation(h1s[:], h1ps[0:K, :], relu, bias=b1_sb[0:K, :])
                    h1sb.append(h1s)
                h2ps = hppool.tile([128, PAIR], f32, tag="hps", name=f"h2ps{p}")
                nc.tensor.matmul(
                    h2ps[0 : 2 * K, :], w2a_sb[0:K, :], h1sb[0][:], start=True, stop=False
                )
                nc.tensor.matmul(
                    h2ps[0 : 2 * K, :], w2b_sb[0:K, :], h1sb[1][:], start=False, stop=True
                )
                h2s = hpool.tile([2 * K, PAIR], hdt, tag="h2sb", name=f"h2sb{p}")
                nc.scalar.activation(h2s[:], h2ps[0 : 2 * K, :], relu, bias=b2_sb[0 : 2 * K, :])
                return h2s

            def emit_g_plain(p, h2s, i):
                gps = gppool.tile([128, 1], f32, tag="g", name=f"g{p}_{i}")
                nc.tensor.matmul(
                    gps[:], h2s[:, i * 128 : (i + 1) * 128], w3_sb[0 : 2 * K, :]
                )
                gs = accpool.tile([128, 1], f32, tag="acc", name=f"gsb{p}_{i}")
                nc.scalar.copy(gs[:], gps[:])
                return gs

            def emit_mlp1_strips(p, x1t_t, x2t_t, ns):
                """ns-way column-tiled MLP-1; strip j handles an n-slice of
                PAIR/ns rows at partitions [32j:32j+8); everything stays
                strip-local. Returns h1sb; the caller emits h2/g later so
                they interleave with the big-matmul stream."""
                sw = PAIR // ns  # n-slice width per strip
                h1sb = []
                for j, xt_t in enumerate((x1t_t, x2t_t)):
                    h1ps = hppool.tile([128, sw], f32, tag="hps", name=f"h1ps{p}_{j}")
                    for c in range(NCHUNK):
                        for st in range(ns):
                            nc.tensor.matmul(
                                h1ps[32 * st : 32 * st + K, :],
                                w1_sb[:, c * K : (c + 1) * K],
                                xt_t[:, c * PAIR + st * sw : c * PAIR + (st + 1) * sw],
                                start=(c == 0),
                                stop=(c == NCHUNK - 1),
                                tile_position=(0, 32 * st),
                            )
                    h1s = hpool.tile([128, sw], hdt, tag="h1sb", name=f"h1sb{p}_{j}")
                    # one relu over all strips; inter-strip rows are dead
                    nc.scalar.activation(
                        h1s[0 : 32 * (ns - 1) + K, :],
                        h1ps[0 : 32 * (ns - 1) + K, :],
                        relu,
                        bias=b1_sb[0 : 32 * (ns - 1) + K, :],
                    )
                    h1sb.append(h1s)
                return h1sb

            def emit_h2_strips(p, h1sb, ns):
                sw = PAIR // ns
                h2ps = hppool.tile([128, sw], f32, tag="hps", name=f"h2ps{p}")
                for st in range(ns):
                    nc.tensor.matmul(
                        h2ps[32 * st : 32 * st + 2 * K, :],
                        w2a_sb[32 * st : 32 * st + K, :],
                        h1sb[0][32 * st : 32 * st + K, :],
                        start=True,
                        stop=False,
                        tile_position=(32 * st, 32 * st),
                    )
                    nc.tensor.matmul(
                        h2ps[32 * st : 32 * st + 2 * K, :],
                        w2b_sb[32 * st : 32 * st + K, :],
                        h1sb[1][32 * st : 32 * st + K, :],
                        start=False,
                        stop=True,
                        tile_position=(32 * st, 32 * st),
                    )
                h2s = hpool.tile([128, sw], hdt, tag="h2sb", name=f"h2sb{p}")
                nc.scalar.activation(
                    h2s[0 : 32 * (ns - 1) + 2 * K, :],
                    h2ps[0 : 32 * (ns - 1) + 2 * K, :],
                    relu,
                    bias=b2_sb[0 : 32 * (ns - 1) + 2 * K, :],
                )
                return h2s

            def emit_g_strip(p, h2s, i, ns):
                ntl = PAIR // 128
                st = i * ns // ntl  # strip holding row-tile i
                off = (i - st * (ntl // ns)) * 128
                gps = gppool.tile([128, 1], f32, tag="g", name=f"g{p}_{i}")
                nc.tensor.matmul(
                    gps[:],
                    h2s[32 * st : 32 * st + 2 * K, off : off + 128],
                    w3_sb[32 * st : 32 * st + 2 * K, :],
                    tile_position=(32 * st, 0),
                )
                gs = accpool.tile([128, 1], f32, tag="acc", name=f"gsb{p}_{i}")
                nc.scalar.copy(gs[:], gps[:])
                return gs

            pre = {}
            if computeonly:
                pre["xx"] = cpool.tile([128, XW], mdt, name="pre_xx")
                nc.sync.dma_start(pre["xx"][:], xx_d[0, :, :])
            if dmaonly or nodot:
                zst = cpool.tile([128, PAIR // 128], f32, name="zst")
                nc.vector.memset(zst[:], 0.0)

            def emit_pair(p):
                if empty:
                    return
                pp = p % npairs
                deng = nc.sync if pp % 2 == 0 else nc.scalar
                if computeonly:
                    xx_t = pre["xx"]
                else:
                    xx_t = xxpool.tile([128, XW], mdt, tag="xx")
                    deng.dma_start(xx_t[:], xx_d[pp, :, :])
                nt = PAIR // 128
                x1_t = xx_t[:, 0 : nt * D]
                x1t_t = xx_t[:, nt * D : 2 * nt * D]
                x2t_t = xx_t[:, 2 * nt * D : 3 * nt * D]
                if dmaonly:
                    deng.dma_start(out_d[pp, :, :], zst[:])
                    return

                h1sb = None
                if nomlp:
                    gsb = None
                elif mlp_mode.startswith("strips"):
                    ns = 2 if mlp_mode == "strips2" else 4
                    h1sb = emit_mlp1_strips(p, x1t_t, x2t_t, ns)
                    emit_g = lambda p_, h_, i_: emit_g_strip(p_, h_, i_, ns)
                    gsb = [None] * (PAIR // 128)
                else:
                    h2s = emit_mlp_plain(p, x1t_t, x2t_t)
                    emit_g = emit_g_plain
                    gsb = [None] * (PAIR // 128)

                stage = None if nodot else stpool.tile([128, PAIR // 128], f32, tag="stage")
                accs = []
                for i in range(PAIR // 128):
                    # P = x2 @ (-S) for this 128-row tile: two 512-col segments
                    pps = [
                        ppool.tile([128, 512], f32, tag="P", name=f"P{p}_{i}_{s_}")
                        for s_ in range(2)
                    ]
                    # seg-major: 8 consecutive MMs per PSUM bank (bank switch 1x
                    # per group, not per MM) to avoid PSUM-queue depth-cycling
                    for seg in range(2):
                        for c in range(NCHUNK):
                            lhs = x2t_t[:, c * PAIR + i * 128 : c * PAIR + (i + 1) * 128]
                            nc.tensor.matmul(
                                pps[seg][:],
                                lhs,
                                s_sb[:, c * D + seg * 512 : c * D + (seg + 1) * 512],
                                start=(c == 0),
                                stop=(c == NCHUNK - 1),
                            )
                    # lag the small matmuls one tile behind the big stream so
                    # the PE never waits on the scalar-engine relu chain:
                    # [big t0][h2][big t1][g0][big t2][g1][big t3][g2][g3]
                    if h1sb is not None and i == 0:
                        h2s = emit_h2_strips(p, h1sb, ns)
                    if gsb is not None and not nodot and i >= 1:
                        gsb[i - 1] = emit_g(p, h2s, i - 1)
                    if nodot:
                        continue
                    # bili: fused mul + rowsum via scalar_tensor_tensor accum
                    acc0 = accpool.tile([128, 1], f32, tag="acc")
                    acc1 = accpool.tile([128, 1], f32, tag="acc")
                    scr0 = scrpool.tile([128, 512], hdt, tag="scr")
                    scr1 = scrpool.tile([128, 512], hdt, tag="scr")
                    nc.vector.scalar_tensor_tensor(
                        scr0[:], x1_t[:, i * D : i * D + 512], 1.0, pps[0][:],
                        op0=mult, op1=mult, accum_out=acc0[:],
                    )
                    nc.vector.scalar_tensor_tensor(
                        scr1[:], x1_t[:, i * D + 512 : (i + 1) * D], 1.0, pps[1][:],
                        op0=mult, op1=mult, accum_out=acc1[:],
                    )
                    accs.append((acc0, acc1))
                if nodot:
                    deng.dma_start(out_d[pp, :, :], zst[:])
                    return
                if gsb is not None:
                    lt = PAIR // 128 - 1
                    gsb[lt] = emit_g(p, h2s, lt)
                for i, (acc0, acc1) in enumerate(accs):
                    if gsb is not None:
                        fin = accpool.tile([128, 1], f32, tag="acc")
                        nc.vector.tensor_add(fin[:], acc0[:], acc1[:])
                        nc.vector.tensor_add(stage[:, i : i + 1], fin[:], gsb[i][:])
                    else:
                        nc.vector.tensor_add(stage[:, i : i + 1], acc0[:], acc1[:])
                deng.dma_start(out_d[pp, :, :], stage[:])

            if reps > 1:
                with tc.For_i(0, reps, 1):
                    for p in range(npairs):
                        emit_pair(p)
            else:
                for p in range(npairs):
                    emit_pair(p)
    nc.compile()
    return nc


def get_program(rows=ROWS, mm_dt=MM_DT):
    key = (rows, mm_dt, MLP_MODE)
    if key not in _PROG_CACHE:
        _PROG_CACHE[key] = _build_program(rows, mm_dt)
    return _PROG_CACHE[key]


def _pack_xt(x, npairs, PAIR, NCHUNK):
    """[rows, D] -> [npairs, 128, NCHUNK*PAIR]; [pair,p,c*PAIR+r] = x[pair*PAIR+r, c*128+p]."""
    return x.reshape(npairs, PAIR, NCHUNK, 128).transpose(0, 3, 2, 1).reshape(
        npairs, 128, NCHUNK * PAIR
    )


def _pack_xrow(x, npairs, PAIR):
    """[rows, D] -> [npairs, 128, (PAIR//128)*D]; [pair,p,i*D+d] = x[pair*PAIR+i*128+p, d]."""
    nt = PAIR // 128
    return x.reshape(npairs, nt, 128, D).transpose(0, 2, 1, 3).reshape(
        npairs, 128, nt * D
    )


def prep_host(x1, x2, U, V, alpha, W1, b1, W2, b2, W3, b3, rows=ROWS, mm_dt=MM_DT):
    """Host-side prep: fold U,V,alpha into -S, replicate tiny weights per
    32-partition strip, shard + transpose + bf16-cast x, pack per-pair rows."""
    f64 = np.float64
    Uf = np.asarray(U, f64).transpose(1, 0, 2).reshape(D, K * RANK)
    Vaf = (np.asarray(V, f64) * np.asarray(alpha, f64)[:, None, None])
    Vaf = Vaf.transpose(1, 0, 2).reshape(D, K * RANK)
    A = Uf @ Vaf.T
    s_use = (A.T - A)  # == -S ; bili = rowsum(x1 * (x2 @ s_use))

    import ml_dtypes

    bfnp = np.dtype(ml_dtypes.bfloat16)
    mnp = bfnp if mm_dt == "bf16" else np.dtype(np.float32)

    NCHUNK = D // 128
    PAIR = 1024
    npairs = rows // PAIR
    s_use = np.ascontiguousarray(
        s_use.reshape(NCHUNK, 128, D).transpose(1, 0, 2).reshape(128, NCHUNK * D)
    ).astype(mnp)
    w1 = np.ascontiguousarray(
        np.asarray(W1, f64).reshape(NCHUNK, 128, K).transpose(1, 0, 2).reshape(128, NCHUNK * K)
    ).astype(mnp)

    # strip-replicated tiny weights: strip j occupies partitions [32j, 32j+2K)
    b1r = np.zeros((128, 1), np.float32)
    b2r = np.zeros((128, 1), np.float32)
    w2a = np.zeros((128, 2 * K), np.float64)
    w2b = np.zeros((128, 2 * K), np.float64)
    w3r = np.zeros((128, 1), np.float64)
    W2f = np.asarray(W2, f64)
    W3f = np.asarray(W3, f64).reshape(K)
    b1f = np.asarray(b1, f64).reshape(K)
    b2f = np.asarray(b2, f64).reshape(K)
    for j in range(4):
        sl8 = slice(32 * j, 32 * j + K)
        b1r[sl8, 0] = b1f
        b2r[32 * j : 32 * j + K, 0] = b2f
        b2r[32 * j + K : 32 * j + 2 * K, 0] = b2f
        w2a[sl8, 0:K] = W2f
        w2b[sl8, K : 2 * K] = W2f
        w3r[32 * j : 32 * j + K, 0] = W3f
        w3r[32 * j + K : 32 * j + 2 * K, 0] = -W3f
    w2a = w2a.astype(bfnp)
    w2b = w2b.astype(bfnp)
    w3r = w3r.astype(bfnp)

    x1f = np.asarray(x1, np.float32).reshape(TOTAL_ROWS, D).astype(mnp)
    x2f = np.asarray(x2, np.float32).reshape(TOTAL_ROWS, D).astype(mnp)
    ncores = TOTAL_ROWS // rows
    in_maps = []
    for c in range(ncores):
        sl = slice(c * rows, (c + 1) * rows)
        xx = np.concatenate(
            [
                _pack_xrow(x1f[sl], npairs, PAIR),
                _pack_xt(x1f[sl], npairs, PAIR, NCHUNK),
                _pack_xt(x2f[sl], npairs, PAIR, NCHUNK),
            ],
            axis=2,
        )
        in_maps.append(
            {
                "xx": np.ascontiguousarray(xx),
                "s": s_use,
                "w1": w1,
                "b1": b1r,
                "b2": b2r,
                "w2a": w2a,
                "w2b": w2b,
                "w3": w3r,
            }
        )
    return in_maps


def unpack_out(raw, rows=ROWS):
    """[npairs, 128, nt] -> [rows]: row pp*1024 + i*128 + p <- raw[pp, p, i]."""
    npairs = rows // 1024
    return np.asarray(raw).transpose(0, 2, 1).reshape(rows)


def run(inputs, trace=False, mm_dt=MM_DT):
    """Run on the 8 NeuronCores. Returns (full_output [B,NR] f32, BassKernelResults)."""
    from concourse.bass_utils import run_bass_kernel_spmd

    in_maps = prep_host(**inputs, rows=ROWS, mm_dt=mm_dt)
    nc = get_program(ROWS, mm_dt)
    res = run_bass_kernel_spmd(nc, in_maps, list(range(NCORES)), trace=trace)
    out = np.concatenate(
        [unpack_out(res.results[c]["out"], ROWS) for c in range(NCORES)]
    )
    return out.reshape(B, NR).astype(np.float32), res


def kernel(**inputs):
    out, _ = run(inputs, trace=False)
    return out


# revision 30
# speedup vs baseline: 1.0935x; 1.0935x over previous
"""Trainium2 Bass kernel for nn_BiLinAntisymmetricFunc.

Math: out[b,n] = g(x1[b,n]) - g(x2[b,n]) + sum_k alpha_k * x1^T (U_k V_k^T - V_k U_k^T) x2

The bilinear part collapses: with S = sum_k alpha_k (U_k V_k^T - V_k U_k^T)
(a precomputable [D,D] weight matrix), bili = x1^T S x2 per row =
rowsum(x1 . (x2 @ (-S))). That is ONE [N,D]@[D,D] matmul instead of four
[N,D]@[D,512] projections (2x fewer FLOPs).

Sharding: data-parallel over the 65536 rows (B*NR) -> 8 cores x 8192 rows.

All matmul operands are bf16 (host-cast): PE runs at the same rate as f32r,
but bf16 halves HBM traffic. Per core we stream x1 (row-major, for the
fused dot), x1^T and x2^T (d-major, for the matmuls) packed into ONE 3MB
DMA per 512-row pair, alternating between the two HWDGE rings (SP/ACT) so
per-transfer fixed costs don't serialize on a single FIFO.

Per 512-row pair, on device:
  - h1^T = relu(W1^T x^T + b1) for x1/x2, 4-way column-tiled
    (tile_position=(0,32j), n-slice j per strip); the MLP chain stays
    strip-local (W2/W3/biases replicated per 32-partition strip).
  - h2cat = relu(W2^T h1 + b2) for both tensors packed into 16 partition
    rows via zero-padded W2 stationaries (x1 -> rows 0:8, x2 -> rows 8:16).
  - gdiff = (h2cat strip)^T @ [W3; -W3] -> [128,1] per row-tile.
  - P = x2 @ (-S): lhsT = x2^T chunks (stationary), rhs = S chunks, f32 PSUM.
  - bili via DVE scalar_tensor_tensor: out=(x1 . P), accum_out=rowsum;
    fin = acc0 + acc1 + gdiff written into a [128,4] staging tile; one
    output DMA per pair (out layout [npairs,128,4], unpermuted on host).
"""

import os

import numpy as np

D, K, RANK = 1024, 8, 64
B, NR = 16, 4096
NCORES = 8
TOTAL_ROWS = B * NR
ROWS = TOTAL_ROWS // NCORES  # 8192 rows per core

MM_DT = os.environ.get("BILIN_MM_DT", "bf16")
MLP_MODE = os.environ.get("BILIN_MLP", "strips")  # "strips" | "plain"

_PROG_CACHE = {}


def _build_program(rows, mm_dt, variant=None, reps=1):
    variant = variant if variant is not None else os.environ.get("BILIN_VARIANT", "")
    mlp_mode = MLP_MODE
    if "plain" in variant:
        mlp_mode = "plain"
    if "strips" in variant:
        mlp_mode = "strips"
    if "strips2" in variant:
        mlp_mode = "strips2"
    nomlp = "nomlp" in variant
    dmaonly = "dmaonly" in variant
    computeonly = "computeonly" in variant
    nodot = "nodot" in variant
    empty = "empty" in variant
    if dmaonly or nodot:
        nomlp = True
    import concourse.bacc as bacc
    import concourse.bass as bass
    import concourse.mybir as mybir
    import concourse.tile as tile

    f32 = mybir.dt.float32
    bf16 = mybir.dt.bfloat16
    if mm_dt == "bf16":
        mdt = bf16
    elif mm_dt == "f32r":
        mdt = mybir.dt.float32r
    else:
        mdt = f32
    hdt = bf16  # tiny MLP h-chain dtype (negligible output contribution)

    nc = bacc.Bacc("TRN2", target_bir_lowering=False, debug=False)

    NCHUNK = D // 128  # 8 contraction chunks
    PAIR = 512
    npairs = rows // PAIR
    XW = 12 * 1024  # packed row: [x1 | x1t | x2t], 4K cols each

    xx_d = nc.dram_tensor("xx", [npairs, 128, XW], mdt, kind="ExternalInput")
    s_d = nc.dram_tensor("s", [128, NCHUNK * D], mdt, kind="ExternalInput")  # packed -S
    w1_d = nc.dram_tensor("w1", [128, NCHUNK * K], mdt, kind="ExternalInput")
    b1_d = nc.dram_tensor("b1", [128, 1], f32, kind="ExternalInput")
    b2_d = nc.dram_tensor("b2", [128, 1], f32, kind="ExternalInput")
    w2a_d = nc.dram_tensor("w2a", [128, 2 * K], hdt, kind="ExternalInput")  # [W2|0]
    w2b_d = nc.dram_tensor("w2b", [128, 2 * K], hdt, kind="ExternalInput")  # [0|W2]
    w3_d = nc.dram_tensor("w3", [128, 1], hdt, kind="ExternalInput")  # [W3;-W3] per strip
    out_d = nc.dram_tensor("out", [npairs, 128, 4], f32, kind="ExternalOutput")

    relu = mybir.ActivationFunctionType.Relu
    mult = mybir.AluOpType.mult

    with tile.TileContext(nc) as tc:
        with (
            tc.tile_pool(name="const", bufs=1) as cpool,
            tc.tile_pool(name="xx", bufs=3) as xxpool,
            tc.tile_pool(name="hsb", bufs=6) as hpool,
            tc.tile_pool(name="scr", bufs=2) as scrpool,
            tc.tile_pool(name="acc", bufs=16) as accpool,
            tc.tile_pool(name="st", bufs=3) as stpool,
            tc.tile_pool(name="pp", bufs=5, space="PSUM") as ppool,
            tc.tile_pool(name="hp", bufs=2, space="PSUM") as hppool,
            tc.tile_pool(name="gp", bufs=1, space="PSUM") as gppool,
        ):
            # ---- resident constants ----
            s_sb = cpool.tile([128, NCHUNK * D], mdt)
            nc.sync.dma_start(s_sb[:], s_d[:, :])
            w1_sb = cpool.tile([128, NCHUNK * K], mdt)
            nc.sync.dma_start(w1_sb[:], w1_d[:, :])
            w2a_sb = cpool.tile([128, 2 * K], hdt)
            nc.sync.dma_start(w2a_sb[:], w2a_d[:, :])
            w2b_sb = cpool.tile([128, 2 * K], hdt)
            nc.sync.dma_start(w2b_sb[:], w2b_d[:, :])
            w3_sb = cpool.tile([128, 1], hdt)
            nc.sync.dma_start(w3_sb[:], w3_d[:, :])
            b1_sb = cpool.tile([128, 1], f32)
            nc.sync.dma_start(b1_sb[:], b1_d[:, :])
            b2_sb = cpool.tile([128, 1], f32)
            nc.sync.dma_start(b2_sb[:], b2_d[:, :])

            def emit_mlp_plain(p, x1t_t, x2t_t):
                """h chains with k on partitions [0:16); h2cat [16, PAIR] sbuf."""
                h1sb = []
                for j, xt_t in enumerate((x1t_t, x2t_t)):
                    h1ps = hppool.tile([128, PAIR], f32, tag="hps", name=f"h1ps{p}_{j}")
                    for c in range(NCHUNK):
                        nc.tensor.matmul(
                            h1ps[0:K, :],
                            w1_sb[:, c * K : (c + 1) * K],
                            xt_t[:, c * PAIR : (c + 1) * PAIR],
                            start=(c == 0),
                            stop=(c == NCHUNK - 1),
                        )
                    h1s = hpool.tile([K, PAIR], hdt, tag="h1sb", name=f"h1sb{p}_{j}")
                    nc.scalar.activation(h1s[:], h1ps[0:K, :], relu, bias=b1_sb[0:K, :])
                    h1sb.append(h1s)
                h2ps = hppool.tile([128, PAIR], f32, tag="hps", name=f"h2ps{p}")
                nc.tensor.matmul(
                    h2ps[0 : 2 * K, :], w2a_sb[0:K, :], h1sb[0][:], start=True, stop=False
                )
                nc.tensor.matmul(
                    h2ps[0 : 2 * K, :], w2b_sb[0:K, :], h1sb[1][:], start=False, stop=True
                )
                h2s = hpool.tile([2 * K, PAIR], hdt, tag="h2sb", name=f"h2sb{p}")
                nc.scalar.activation(h2s[:], h2ps[0 : 2 * K, :], relu, bias=b2_sb[0 : 2 * K, :])
                return h2s

            def emit_g_plain(p, h2s, i):
                gps = gppool.tile([128, 1], f32, tag="g", name=f"g{p}_{i}")
                nc.tensor.matmul(
                    gps[:], h2s[:, i * 128 : (i + 1) * 128], w3_sb[0 : 2 * K, :]
                )
                gs = accpool.tile([128, 1], f32, tag="acc", name=f"gsb{p}_{i}")
                nc.scalar.copy(gs[:], gps[:])
                return gs

            def emit_mlp1_strips(p, x1t_t, x2t_t, ns):
                """ns-way column-tiled MLP-1; strip j handles an n-slice of
                PAIR/ns rows at partitions [32j:32j+8); everything stays
                strip-local. Returns h1sb; the caller emits h2/g later so
                they interleave with the big-matmul stream."""
                sw = PAIR // ns  # n-slice width per strip
                h1sb = []
                for j, xt_t in enumerate((x1t_t, x2t_t)):
                    h1ps = hppool.tile([128, sw], f32, tag="hps", name=f"h1ps{p}_{j}")
                    for c in range(NCHUNK):
                        for st in range(ns):
                            nc.tensor.matmul(
                                h1ps[32 * st : 32 * st + K, :],
                                w1_sb[:, c * K : (c + 1) * K],
                                xt_t[:, c * PAIR + st * sw : c * PAIR + (st + 1) * sw],
                                start=(c == 0),
                                stop=(c == NCHUNK - 1),
                                tile_position=(0, 32 * st),
                            )
                    h1s = hpool.tile([128, sw], hdt, tag="h1sb", name=f"h1sb{p}_{j}")
                    # one relu over all strips; inter-strip rows are dead
                    nc.scalar.activation(
                        h1s[0 : 32 * (ns - 1) + K, :],
                        h1ps[0 : 32 * (ns - 1) + K, :],
                        relu,
                        bias=b1_sb[0 : 32 * (ns - 1) + K, :],
                    )
                    h1sb.append(h1s)
                return h1sb

            def emit_h2_strips(p, h1sb, ns):
                sw = PAIR // ns
                h2ps = hppool.tile([128, sw], f32, tag="hps", name=f"h2ps{p}")
                for st in range(ns):
                    nc.tensor.matmul(
                        h2ps[32 * st : 32 * st + 2 * K, :],
                        w2a_sb[32 * st : 32 * st + K, :],
                        h1sb[0][32 * st : 32 * st + K, :],
                        start=True,
                        stop=False,
                        tile_position=(32 * st, 32 * st),
                    )
                    nc.tensor.matmul(
                        h2ps[32 * st : 32 * st + 2 * K, :],
                        w2b_sb[32 * st : 32 * st + K, :],
                        h1sb[1][32 * st : 32 * st + K, :],
                        start=False,
                        stop=True,
                        tile_position=(32 * st, 32 * st),
                    )
                h2s = hpool.tile([128, sw], hdt, tag="h2sb", name=f"h2sb{p}")
                nc.scalar.activation(
                    h2s[0 : 32 * (ns - 1) + 2 * K, :],
                    h2ps[0 : 32 * (ns - 1) + 2 * K, :],
                    relu,
                    bias=b2_sb[0 : 32 * (ns - 1) + 2 * K, :],
                )
                return h2s

            def emit_g_strip(p, h2s, i, ns):
                st = i * ns // 4  # strip holding row-tile i
                off = (i - st * (4 // ns)) * 128
                gps = gppool.tile([128, 1], f32, tag="g", name=f"g{p}_{i}")
                nc.tensor.matmul(
                    gps[:],
                    h2s[32 * st : 32 * st + 2 * K, off : off + 128],
                    w3_sb[32 * st : 32 * st + 2 * K, :],
                    tile_position=(32 * st, 0),
                )
                gs = accpool.tile([128, 1], f32, tag="acc", name=f"gsb{p}_{i}")
                nc.scalar.copy(gs[:], gps[:])
                return gs

            pre = {}
            if computeonly:
                pre["xx"] = cpool.tile([128, XW], mdt, name="pre_xx")
                nc.sync.dma_start(pre["xx"][:], xx_d[0, :, :])
            if dmaonly or nodot:
                zst = cpool.tile([128, 4], f32, name="zst")
                nc.vector.memset(zst[:], 0.0)

            def emit_pair(p):
                if empty:
                    return
                pp = p % npairs
                deng = nc.sync if pp % 2 == 0 else nc.scalar
                if computeonly:
                    xx_t = pre["xx"]
                else:
                    xx_t = xxpool.tile([128, XW], mdt, tag="xx")
                    deng.dma_start(xx_t[:], xx_d[pp, :, :])
                x1_t = xx_t[:, 0 : 4 * D]
                x1t_t = xx_t[:, 4 * D : 8 * D]
                x2t_t = xx_t[:, 8 * D : 12 * D]
                if dmaonly:
                    deng.dma_start(out_d[pp, :, :], zst[:])
                    return

                h1sb = None
                if nomlp:
                    gsb = None
                elif mlp_mode.startswith("strips"):
                    ns = 2 if mlp_mode == "strips2" else 4
                    h1sb = emit_mlp1_strips(p, x1t_t, x2t_t, ns)
                    emit_g = lambda p_, h_, i_: emit_g_strip(p_, h_, i_, ns)
                    gsb = [None] * 4
                else:
                    h2s = emit_mlp_plain(p, x1t_t, x2t_t)
                    emit_g = emit_g_plain
                    gsb = [None] * 4

                stage = None if nodot else stpool.tile([128, 4], f32, tag="stage")
                accs = []
                for i in range(PAIR // 128):
                    # P = x2 @ (-S) for this 128-row tile: two 512-col segments
                    pps = [
                        ppool.tile([128, 512], f32, tag="P", name=f"P{p}_{i}_{s_}")
                        for s_ in range(2)
                    ]
                    # seg-major: 8 consecutive MMs per PSUM bank (bank switch 1x
                    # per group, not per MM) to avoid PSUM-queue depth-cycling
                    for seg in range(2):
                        for c in range(NCHUNK):
                            lhs = x2t_t[:, c * PAIR + i * 128 : c * PAIR + (i + 1) * 128]
                            nc.tensor.matmul(
                                pps[seg][:],
                                lhs,
                                s_sb[:, c * D + seg * 512 : c * D + (seg + 1) * 512],
                                start=(c == 0),
                                stop=(c == NCHUNK - 1),
                            )
                    # lag the small matmuls one tile behind the big stream so
                    # the PE never waits on the scalar-engine relu chain:
                    # [big t0][h2][big t1][g0][big t2][g1][big t3][g2][g3]
                    if h1sb is not None and i == 0:
                        h2s = emit_h2_strips(p, h1sb, ns)
                    if gsb is not None and not nodot and i >= 1:
                        gsb[i - 1] = emit_g(p, h2s, i - 1)
                    if nodot:
                        continue
                    # bili: fused mul + rowsum via scalar_tensor_tensor accum
                    acc0 = accpool.tile([128, 1], f32, tag="acc")
                    acc1 = accpool.tile([128, 1], f32, tag="acc")
                    scr0 = scrpool.tile([128, 512], hdt, tag="scr")
                    scr1 = scrpool.tile([128, 512], hdt, tag="scr")
                    nc.vector.scalar_tensor_tensor(
                        scr0[:], x1_t[:, i * D : i * D + 512], 1.0, pps[0][:],
                        op0=mult, op1=mult, accum_out=acc0[:],
                    )
                    nc.vector.scalar_tensor_tensor(
                        scr1[:], x1_t[:, i * D + 512 : (i + 1) * D], 1.0, pps[1][:],
                        op0=mult, op1=mult, accum_out=acc1[:],
                    )
                    accs.append((acc0, acc1))
                if nodot:
                    deng.dma_start(out_d[pp, :, :], zst[:])
                    return
                if gsb is not None:
                    gsb[3] = emit_g(p, h2s, 3)
                for i, (acc0, acc1) in enumerate(accs):
                    if gsb is not None:
                        fin = accpool.tile([128, 1], f32, tag="acc")
                        nc.vector.tensor_add(fin[:], acc0[:], acc1[:])
                        nc.vector.tensor_add(stage[:, i : i + 1], fin[:], gsb[i][:])
                    else:
                        nc.vector.tensor_add(stage[:, i : i + 1], acc0[:], acc1[:])
                deng.dma_start(out_d[pp, :, :], stage[:])

            if reps > 1:
                with tc.For_i(0, reps, 1):
                    for p in range(npairs):
                        emit_pair(p)
            else:
                for p in range(npairs):
                    emit_pair(p)
    nc.compile()
    return nc


def get_program(rows=ROWS, mm_dt=MM_DT):
    key = (rows, mm_dt, MLP_MODE)
    if key not in _PROG_CACHE:
        _PROG_CACHE[key] = _build_program(rows, mm_dt)
    return _PROG_CACHE[key]


def _pack_xt(x, npairs, PAIR, NCHUNK):
    """[rows, D] -> [npairs, 128, NCHUNK*PAIR]; [pair,p,c*PAIR+r] = x[pair*PAIR+r, c*128+p]."""
    return x.reshape(npairs, PAIR, NCHUNK, 128).transpose(0, 3, 2, 1).reshape(
        npairs, 128, NCHUNK * PAIR
    )


def _pack_xrow(x, npairs, PAIR):
    """[rows, D] -> [npairs, 128, (PAIR//128)*D]; [pair,p,i*D+d] = x[pair*PAIR+i*128+p, d]."""
    nt = PAIR // 128
    return x.reshape(npairs, nt, 128, D).transpose(0, 2, 1, 3).reshape(
        npairs, 128, nt * D
    )


def prep_host(x1, x2, U, V, alpha, W1, b1, W2, b2, W3, b3, rows=ROWS, mm_dt=MM_DT):
    """Host-side prep: fold U,V,alpha into -S, replicate tiny weights per
    32-partition strip, shard + transpose + bf16-cast x, pack per-pair rows."""
    f64 = np.float64
    Uf = np.asarray(U, f64).transpose(1, 0, 2).reshape(D, K * RANK)
    Vaf = (np.asarray(V, f64) * np.asarray(alpha, f64)[:, None, None])
    Vaf = Vaf.transpose(1, 0, 2).reshape(D, K * RANK)
    A = Uf @ Vaf.T
    s_use = (A.T - A)  # == -S ; bili = rowsum(x1 * (x2 @ s_use))

    import ml_dtypes

    bfnp = np.dtype(ml_dtypes.bfloat16)
    mnp = bfnp if mm_dt == "bf16" else np.dtype(np.float32)

    NCHUNK = D // 128
    PAIR = 512
    npairs = rows // PAIR
    s_use = np.ascontiguousarray(
        s_use.reshape(NCHUNK, 128, D).transpose(1, 0, 2).reshape(128, NCHUNK * D)
    ).astype(mnp)
    w1 = np.ascontiguousarray(
        np.asarray(W1, f64).reshape(NCHUNK, 128, K).transpose(1, 0, 2).reshape(128, NCHUNK * K)
    ).astype(mnp)

    # strip-replicated tiny weights: strip j occupies partitions [32j, 32j+2K)
    b1r = np.zeros((128, 1), np.float32)
    b2r = np.zeros((128, 1), np.float32)
    w2a = np.zeros((128, 2 * K), np.float64)
    w2b = np.zeros((128, 2 * K), np.float64)
    w3r = np.zeros((128, 1), np.float64)
    W2f = np.asarray(W2, f64)
    W3f = np.asarray(W3, f64).reshape(K)
    b1f = np.asarray(b1, f64).reshape(K)
    b2f = np.asarray(b2, f64).reshape(K)
    for j in range(4):
        sl8 = slice(32 * j, 32 * j + K)
        b1r[sl8, 0] = b1f
        b2r[32 * j : 32 * j + K, 0] = b2f
        b2r[32 * j + K : 32 * j + 2 * K, 0] = b2f
        w2a[sl8, 0:K] = W2f
        w2b[sl8, K : 2 * K] = W2f
        w3r[32 * j : 32 * j + K, 0] = W3f
        w3r[32 * j + K : 32 * j + 2 * K, 0] = -W3f
    w2a = w2a.astype(bfnp)
    w2b = w2b.astype(bfnp)
    w3r = w3r.astype(bfnp)

    x1f = np.asarray(x1, np.float32).reshape(TOTAL_ROWS, D).astype(mnp)
    x2f = np.asarray(x2, np.float32).reshape(TOTAL_ROWS, D).astype(mnp)
    ncores = TOTAL_ROWS // rows
    in_maps = []
    for c in range(ncores):
        sl = slice(c * rows, (c + 1) * rows)
        xx = np.concatenate(
            [
                _pack_xrow(x1f[sl], npairs, PAIR),
                _pack_xt(x1f[sl], npairs, PAIR, NCHUNK),
                _pack_xt(x2f[sl], npairs, PAIR, NCHUNK),
            ],
            axis=2,
        )
        in_maps.append(
            {
                "xx": np.ascontiguousarray(xx),
                "s": s_use,
                "w1": w1,
                "b1": b1r,
                "b2": b2r,
                "w2a": w2a,
                "w2b": w2b,
                "w3": w3r,
            }
        )
    return in_maps


def unpack_out(raw, rows=ROWS):
    """[npairs, 128, 4] -> [rows]: row pp*512 + i*128 + p <- raw[pp, p, i]."""
    npairs = rows // 512
    return np.asarray(raw).transpose(0, 2, 1).reshape(rows)


def run(inputs, trace=False, mm_dt=MM_DT):
    """Run on the 8 NeuronCores. Returns (full_output [B,NR] f32, BassKernelResults)."""
    from concourse.bass_utils import run_bass_kernel_spmd

    in_maps = prep_host(**inputs, rows=ROWS, mm_dt=mm_dt)
    nc = get_program(ROWS, mm_dt)
    res = run_bass_kernel_spmd(nc, in_maps, list(range(NCORES)), trace=trace)
    out = np.concatenate(
        [unpack_out(res.results[c]["out"], ROWS) for c in range(NCORES)]
    )
    return out.reshape(B, NR).astype(np.float32), res


def kernel(**inputs):
    out, _ = run(inputs, trace=False)
    return out


# revision 31
# speedup vs baseline: 1.0996x; 1.0055x over previous
"""Trainium2 Bass kernel for nn_BiLinAntisymmetricFunc.

Math: out[b,n] = g(x1[b,n]) - g(x2[b,n]) + sum_k alpha_k * x1^T (U_k V_k^T - V_k U_k^T) x2

The bilinear part collapses: with S = sum_k alpha_k (U_k V_k^T - V_k U_k^T)
(a precomputable [D,D] weight matrix), bili = x1^T S x2 per row =
rowsum(x1 . (x2 @ (-S))). That is ONE [N,D]@[D,D] matmul instead of four
[N,D]@[D,512] projections (2x fewer FLOPs).

Sharding: data-parallel over the 65536 rows (B*NR) -> 8 cores x 8192 rows.

All matmul operands are bf16 (host-cast): PE runs at the same rate as f32r,
but bf16 halves HBM traffic. Per core we stream x1 (row-major, for the
fused dot), x1^T and x2^T (d-major, for the matmuls) packed into ONE 3MB
DMA per 512-row pair, alternating between the two HWDGE rings (SP/ACT) so
per-transfer fixed costs don't serialize on a single FIFO.

Per 512-row pair, on device:
  - h1^T = relu(W1^T x^T + b1) for x1/x2, 4-way column-tiled
    (tile_position=(0,32j), n-slice j per strip); the MLP chain stays
    strip-local (W2/W3/biases replicated per 32-partition strip).
  - h2cat = relu(W2^T h1 + b2) for both tensors packed into 16 partition
    rows via zero-padded W2 stationaries (x1 -> rows 0:8, x2 -> rows 8:16).
  - gdiff = (h2cat strip)^T @ [W3; -W3] -> [128,1] per row-tile.
  - P = x2 @ (-S): lhsT = x2^T chunks (stationary), rhs = S chunks, f32 PSUM.
  - bili via DVE scalar_tensor_tensor: out=(x1 . P), accum_out=rowsum;
    fin = acc0 + acc1 + gdiff written into a [128,4] staging tile; one
    output DMA per pair (out layout [npairs,128,4], unpermuted on host).
"""

import os

import numpy as np

D, K, RANK = 1024, 8, 64
B, NR = 16, 4096
NCORES = 8
TOTAL_ROWS = B * NR
ROWS = TOTAL_ROWS // NCORES  # 8192 rows per core

MM_DT = os.environ.get("BILIN_MM_DT", "bf16")
MLP_MODE = os.environ.get("BILIN_MLP", "strips")  # "strips" | "plain"

_PROG_CACHE = {}


def _build_program(rows, mm_dt, variant=None, reps=1):
    variant = variant if variant is not None else os.environ.get("BILIN_VARIANT", "")
    mlp_mode = MLP_MODE
    if "plain" in variant:
        mlp_mode = "plain"
    if "strips" in variant:
        mlp_mode = "strips"
    if "strips2" in variant:
        mlp_mode = "strips2"
    nomlp = "nomlp" in variant
    dmaonly = "dmaonly" in variant
    computeonly = "computeonly" in variant
    nodot = "nodot" in variant
    empty = "empty" in variant
    if dmaonly or nodot:
        nomlp = True
    import concourse.bacc as bacc
    import concourse.bass as bass
    import concourse.mybir as mybir
    import concourse.tile as tile

    f32 = mybir.dt.float32
    bf16 = mybir.dt.bfloat16
    if mm_dt == "bf16":
        mdt = bf16
    elif mm_dt == "f32r":
        mdt = mybir.dt.float32r
    else:
        mdt = f32
    hdt = bf16  # tiny MLP h-chain dtype (negligible output contribution)

    nc = bacc.Bacc("TRN2", target_bir_lowering=False, debug=False)

    NCHUNK = D // 128  # 8 contraction chunks
    PAIR = 512
    npairs = rows // PAIR
    XW = 12 * 1024  # packed row: [x1 | x1t | x2t], 4K cols each

    xx_d = nc.dram_tensor("xx", [npairs, 128, XW], mdt, kind="ExternalInput")
    s_d = nc.dram_tensor("s", [128, NCHUNK * D], mdt, kind="ExternalInput")  # packed -S
    w1_d = nc.dram_tensor("w1", [128, NCHUNK * K], mdt, kind="ExternalInput")
    b1_d = nc.dram_tensor("b1", [128, 1], f32, kind="ExternalInput")
    b2_d = nc.dram_tensor("b2", [128, 1], f32, kind="ExternalInput")
    w2a_d = nc.dram_tensor("w2a", [128, 2 * K], hdt, kind="ExternalInput")  # [W2|0]
    w2b_d = nc.dram_tensor("w2b", [128, 2 * K], hdt, kind="ExternalInput")  # [0|W2]
    w3_d = nc.dram_tensor("w3", [128, 1], hdt, kind="ExternalInput")  # [W3;-W3] per strip
    out_d = nc.dram_tensor("out", [npairs, 128, 4], f32, kind="ExternalOutput")

    relu = mybir.ActivationFunctionType.Relu
    mult = mybir.AluOpType.mult

    with tile.TileContext(nc) as tc:
        with (
            tc.tile_pool(name="const", bufs=1) as cpool,
            tc.tile_pool(name="xx", bufs=3) as xxpool,
            tc.tile_pool(name="hsb", bufs=6) as hpool,
            tc.tile_pool(name="scr", bufs=2) as scrpool,
            tc.tile_pool(name="acc", bufs=16) as accpool,
            tc.tile_pool(name="st", bufs=3) as stpool,
            tc.tile_pool(name="pp", bufs=5, space="PSUM") as ppool,
            tc.tile_pool(name="hp", bufs=2, space="PSUM") as hppool,
            tc.tile_pool(name="gp", bufs=1, space="PSUM") as gppool,
        ):
            # ---- resident constants ----
            s_sb = cpool.tile([128, NCHUNK * D], mdt)
            nc.scalar.dma_start(s_sb[:], s_d[:, :])
            w1_sb = cpool.tile([128, NCHUNK * K], mdt)
            nc.scalar.dma_start(w1_sb[:], w1_d[:, :])
            w2a_sb = cpool.tile([128, 2 * K], hdt)
            nc.sync.dma_start(w2a_sb[:], w2a_d[:, :])
            w2b_sb = cpool.tile([128, 2 * K], hdt)
            nc.sync.dma_start(w2b_sb[:], w2b_d[:, :])
            w3_sb = cpool.tile([128, 1], hdt)
            nc.sync.dma_start(w3_sb[:], w3_d[:, :])
            b1_sb = cpool.tile([128, 1], f32)
            nc.sync.dma_start(b1_sb[:], b1_d[:, :])
            b2_sb = cpool.tile([128, 1], f32)
            nc.sync.dma_start(b2_sb[:], b2_d[:, :])

            def emit_mlp_plain(p, x1t_t, x2t_t):
                """h chains with k on partitions [0:16); h2cat [16, PAIR] sbuf."""
                h1sb = []
                for j, xt_t in enumerate((x1t_t, x2t_t)):
                    h1ps = hppool.tile([128, PAIR], f32, tag="hps", name=f"h1ps{p}_{j}")
                    for c in range(NCHUNK):
                        nc.tensor.matmul(
                            h1ps[0:K, :],
                            w1_sb[:, c * K : (c + 1) * K],
                            xt_t[:, c * PAIR : (c + 1) * PAIR],
                            start=(c == 0),
                            stop=(c == NCHUNK - 1),
                        )
                    h1s = hpool.tile([K, PAIR], hdt, tag="h1sb", name=f"h1sb{p}_{j}")
                    nc.scalar.activation(h1s[:], h1ps[0:K, :], relu, bias=b1_sb[0:K, :])
                    h1sb.append(h1s)
                h2ps = hppool.tile([128, PAIR], f32, tag="hps", name=f"h2ps{p}")
                nc.tensor.matmul(
                    h2ps[0 : 2 * K, :], w2a_sb[0:K, :], h1sb[0][:], start=True, stop=False
                )
                nc.tensor.matmul(
                    h2ps[0 : 2 * K, :], w2b_sb[0:K, :], h1sb[1][:], start=False, stop=True
                )
                h2s = hpool.tile([2 * K, PAIR], hdt, tag="h2sb", name=f"h2sb{p}")
                nc.scalar.activation(h2s[:], h2ps[0 : 2 * K, :], relu, bias=b2_sb[0 : 2 * K, :])
                return h2s

            def emit_g_plain(p, h2s, i):
                gps = gppool.tile([128, 1], f32, tag="g", name=f"g{p}_{i}")
                nc.tensor.matmul(
                    gps[:], h2s[:, i * 128 : (i + 1) * 128], w3_sb[0 : 2 * K, :]
                )
                gs = accpool.tile([128, 1], f32, tag="acc", name=f"gsb{p}_{i}")
                nc.scalar.copy(gs[:], gps[:])
                return gs

            def emit_mlp1_strips(p, x1t_t, x2t_t, ns):
                """ns-way column-tiled MLP-1; strip j handles an n-slice of
                PAIR/ns rows at partitions [32j:32j+8); everything stays
                strip-local. Returns h1sb; the caller emits h2/g later so
                they interleave with the big-matmul stream."""
                sw = PAIR // ns  # n-slice width per strip
                h1sb = []
                for j, xt_t in enumerate((x1t_t, x2t_t)):
                    h1ps = hppool.tile([128, sw], f32, tag="hps", name=f"h1ps{p}_{j}")
                    for c in range(NCHUNK):
                        for st in range(ns):
                            nc.tensor.matmul(
                                h1ps[32 * st : 32 * st + K, :],
                                w1_sb[:, c * K : (c + 1) * K],
                                xt_t[:, c * PAIR + st * sw : c * PAIR + (st + 1) * sw],
                                start=(c == 0),
                                stop=(c == NCHUNK - 1),
                                tile_position=(0, 32 * st),
                            )
                    h1s = hpool.tile([128, sw], hdt, tag="h1sb", name=f"h1sb{p}_{j}")
                    # one relu over all strips; inter-strip rows are dead
                    nc.scalar.activation(
                        h1s[0 : 32 * (ns - 1) + K, :],
                        h1ps[0 : 32 * (ns - 1) + K, :],
                        relu,
                        bias=b1_sb[0 : 32 * (ns - 1) + K, :],
                    )
                    h1sb.append(h1s)
                return h1sb

            def emit_h2_strips(p, h1sb, ns):
                sw = PAIR // ns
                h2ps = hppool.tile([128, sw], f32, tag="hps", name=f"h2ps{p}")
                for st in range(ns):
                    nc.tensor.matmul(
                        h2ps[32 * st : 32 * st + 2 * K, :],
                        w2a_sb[32 * st : 32 * st + K, :],
                        h1sb[0][32 * st : 32 * st + K, :],
                        start=True,
                        stop=False,
                        tile_position=(32 * st, 32 * st),
                    )
                    nc.tensor.matmul(
                        h2ps[32 * st : 32 * st + 2 * K, :],
                        w2b_sb[32 * st : 32 * st + K, :],
                        h1sb[1][32 * st : 32 * st + K, :],
                        start=False,
                        stop=True,
                        tile_position=(32 * st, 32 * st),
                    )
                h2s = hpool.tile([128, sw], hdt, tag="h2sb", name=f"h2sb{p}")
                nc.scalar.activation(
                    h2s[0 : 32 * (ns - 1) + 2 * K, :],
                    h2ps[0 : 32 * (ns - 1) + 2 * K, :],
                    relu,
                    bias=b2_sb[0 : 32 * (ns - 1) + 2 * K, :],
                )
                return h2s

            def emit_g_strip(p, h2s, i, ns):
                st = i * ns // 4  # strip holding row-tile i
                off = (i - st * (4 // ns)) * 128
                gps = gppool.tile([128, 1], f32, tag="g", name=f"g{p}_{i}")
                nc.tensor.matmul(
                    gps[:],
                    h2s[32 * st : 32 * st + 2 * K, off : off + 128],
                    w3_sb[32 * st : 32 * st + 2 * K, :],
                    tile_position=(32 * st, 0),
                )
                gs = accpool.tile([128, 1], f32, tag="acc", name=f"gsb{p}_{i}")
                nc.scalar.copy(gs[:], gps[:])
                return gs

            pre = {}
            if computeonly:
                pre["xx"] = cpool.tile([128, XW], mdt, name="pre_xx")
                nc.sync.dma_start(pre["xx"][:], xx_d[0, :, :])
            if dmaonly or nodot:
                zst = cpool.tile([128, 4], f32, name="zst")
                nc.vector.memset(zst[:], 0.0)

            def emit_pair(p):
                if empty:
                    return
                pp = p % npairs
                deng = nc.sync if pp % 2 == 0 else nc.scalar
                if computeonly:
                    xx_t = pre["xx"]
                else:
                    xx_t = xxpool.tile([128, XW], mdt, tag="xx")
                    deng.dma_start(xx_t[:], xx_d[pp, :, :])
                x1_t = xx_t[:, 0 : 4 * D]
                x1t_t = xx_t[:, 4 * D : 8 * D]
                x2t_t = xx_t[:, 8 * D : 12 * D]
                if dmaonly:
                    deng.dma_start(out_d[pp, :, :], zst[:])
                    return

                h1sb = None
                if nomlp:
                    gsb = None
                elif mlp_mode.startswith("strips"):
                    ns = 2 if mlp_mode == "strips2" else 4
                    h1sb = emit_mlp1_strips(p, x1t_t, x2t_t, ns)
                    emit_g = lambda p_, h_, i_: emit_g_strip(p_, h_, i_, ns)
                    gsb = [None] * 4
                else:
                    h2s = emit_mlp_plain(p, x1t_t, x2t_t)
                    emit_g = emit_g_plain
                    gsb = [None] * 4

                stage = None if nodot else stpool.tile([128, 4], f32, tag="stage")
                accs = []
                for i in range(PAIR // 128):
                    # P = x2 @ (-S) for this 128-row tile: two 512-col segments
                    pps = [
                        ppool.tile([128, 512], f32, tag="P", name=f"P{p}_{i}_{s_}")
                        for s_ in range(2)
                    ]
                    # seg-major: 8 consecutive MMs per PSUM bank (bank switch 1x
                    # per group, not per MM) to avoid PSUM-queue depth-cycling
                    for seg in range(2):
                        for c in range(NCHUNK):
                            lhs = x2t_t[:, c * PAIR + i * 128 : c * PAIR + (i + 1) * 128]
                            nc.tensor.matmul(
                                pps[seg][:],
                                lhs,
                                s_sb[:, c * D + seg * 512 : c * D + (seg + 1) * 512],
                                start=(c == 0),
                                stop=(c == NCHUNK - 1),
                            )
                    # lag the small matmuls one tile behind the big stream so
                    # the PE never waits on the scalar-engine relu chain:
                    # [big t0][h2][big t1][g0][big t2][g1][big t3][g2][g3]
                    if h1sb is not None and i == 0:
                        h2s = emit_h2_strips(p, h1sb, ns)
                    if gsb is not None and not nodot and i >= 1:
                        gsb[i - 1] = emit_g(p, h2s, i - 1)
                    if nodot:
                        continue
                    # bili: fused mul + rowsum via scalar_tensor_tensor accum
                    acc0 = accpool.tile([128, 1], f32, tag="acc")
                    acc1 = accpool.tile([128, 1], f32, tag="acc")
                    scr0 = scrpool.tile([128, 512], hdt, tag="scr")
                    scr1 = scrpool.tile([128, 512], hdt, tag="scr")
                    nc.vector.scalar_tensor_tensor(
                        scr0[:], x1_t[:, i * D : i * D + 512], 1.0, pps[0][:],
                        op0=mult, op1=mult, accum_out=acc0[:],
                    )
                    nc.vector.scalar_tensor_tensor(
                        scr1[:], x1_t[:, i * D + 512 : (i + 1) * D], 1.0, pps[1][:],
                        op0=mult, op1=mult, accum_out=acc1[:],
                    )
                    accs.append((acc0, acc1))
                if nodot:
                    deng.dma_start(out_d[pp, :, :], zst[:])
                    return
                if gsb is not None:
                    gsb[3] = emit_g(p, h2s, 3)
                for i, (acc0, acc1) in enumerate(accs):
                    if gsb is not None:
                        fin = accpool.tile([128, 1], f32, tag="acc")
                        nc.vector.tensor_add(fin[:], acc0[:], acc1[:])
                        nc.vector.tensor_add(stage[:, i : i + 1], fin[:], gsb[i][:])
                    else:
                        nc.vector.tensor_add(stage[:, i : i + 1], acc0[:], acc1[:])
                deng.dma_start(out_d[pp, :, :], stage[:])

            if reps > 1:
                with tc.For_i(0, reps, 1):
                    for p in range(npairs):
                        emit_pair(p)
            else:
                for p in range(npairs):
                    emit_pair(p)
    nc.compile()
    return nc


def get_program(rows=ROWS, mm_dt=MM_DT):
    key = (rows, mm_dt, MLP_MODE)
    if key not in _PROG_CACHE:
        _PROG_CACHE[key] = _build_program(rows, mm_dt)
    return _PROG_CACHE[key]


def _pack_xt(x, npairs, PAIR, NCHUNK):
    """[rows, D] -> [npairs, 128, NCHUNK*PAIR]; [pair,p,c*PAIR+r] = x[pair*PAIR+r, c*128+p]."""
    return x.reshape(npairs, PAIR, NCHUNK, 128).transpose(0, 3, 2, 1).reshape(
        npairs, 128, NCHUNK * PAIR
    )


def _pack_xrow(x, npairs, PAIR):
    """[rows, D] -> [npairs, 128, (PAIR//128)*D]; [pair,p,i*D+d] = x[pair*PAIR+i*128+p, d]."""
    nt = PAIR // 128
    return x.reshape(npairs, nt, 128, D).transpose(0, 2, 1, 3).reshape(
        npairs, 128, nt * D
    )


def prep_host(x1, x2, U, V, alpha, W1, b1, W2, b2, W3, b3, rows=ROWS, mm_dt=MM_DT):
    """Host-side prep: fold U,V,alpha into -S, replicate tiny weights per
    32-partition strip, shard + transpose + bf16-cast x, pack per-pair rows."""
    f64 = np.float64
    Uf = np.asarray(U, f64).transpose(1, 0, 2).reshape(D, K * RANK)
    Vaf = (np.asarray(V, f64) * np.asarray(alpha, f64)[:, None, None])
    Vaf = Vaf.transpose(1, 0, 2).reshape(D, K * RANK)
    A = Uf @ Vaf.T
    s_use = (A.T - A)  # == -S ; bili = rowsum(x1 * (x2 @ s_use))

    import ml_dtypes

    bfnp = np.dtype(ml_dtypes.bfloat16)
    mnp = bfnp if mm_dt == "bf16" else np.dtype(np.float32)

    NCHUNK = D // 128
    PAIR = 512
    npairs = rows // PAIR
    s_use = np.ascontiguousarray(
        s_use.reshape(NCHUNK, 128, D).transpose(1, 0, 2).reshape(128, NCHUNK * D)
    ).astype(mnp)
    w1 = np.ascontiguousarray(
        np.asarray(W1, f64).reshape(NCHUNK, 128, K).transpose(1, 0, 2).reshape(128, NCHUNK * K)
    ).astype(mnp)

    # strip-replicated tiny weights: strip j occupies partitions [32j, 32j+2K)
    b1r = np.zeros((128, 1), np.float32)
    b2r = np.zeros((128, 1), np.float32)
    w2a = np.zeros((128, 2 * K), np.float64)
    w2b = np.zeros((128, 2 * K), np.float64)
    w3r = np.zeros((128, 1), np.float64)
    W2f = np.asarray(W2, f64)
    W3f = np.asarray(W3, f64).reshape(K)
    b1f = np.asarray(b1, f64).reshape(K)
    b2f = np.asarray(b2, f64).reshape(K)
    for j in range(4):
        sl8 = slice(32 * j, 32 * j + K)
        b1r[sl8, 0] = b1f
        b2r[32 * j : 32 * j + K, 0] = b2f
        b2r[32 * j + K : 32 * j + 2 * K, 0] = b2f
        w2a[sl8, 0:K] = W2f
        w2b[sl8, K : 2 * K] = W2f
        w3r[32 * j : 32 * j + K, 0] = W3f
        w3r[32 * j + K : 32 * j + 2 * K, 0] = -W3f
    w2a = w2a.astype(bfnp)
    w2b = w2b.astype(bfnp)
    w3r = w3r.astype(bfnp)

    x1f = np.asarray(x1, np.float32).reshape(TOTAL_ROWS, D).astype(mnp)
    x2f = np.asarray(x2, np.float32).reshape(TOTAL_ROWS, D).astype(mnp)
    ncores = TOTAL_ROWS // rows
    in_maps = []
    for c in range(ncores):
        sl = slice(c * rows, (c + 1) * rows)
        xx = np.concatenate(
            [
                _pack_xrow(x1f[sl], npairs, PAIR),
                _pack_xt(x1f[sl], npairs, PAIR, NCHUNK),
                _pack_xt(x2f[sl], npairs, PAIR, NCHUNK),
            ],
            axis=2,
        )
        in_maps.append(
            {
                "xx": np.ascontiguousarray(xx),
                "s": s_use,
                "w1": w1,
                "b1": b1r,
                "b2": b2r,
                "w2a": w2a,
                "w2b": w2b,
                "w3": w3r,
            }
        )
    return in_maps


def unpack_out(raw, rows=ROWS):
    """[npairs, 128, 4] -> [rows]: row pp*512 + i*128 + p <- raw[pp, p, i]."""
    npairs = rows // 512
    return np.asarray(raw).transpose(0, 2, 1).reshape(rows)


def run(inputs, trace=False, mm_dt=MM_DT):
    """Run on the 8 NeuronCores. Returns (full_output [B,NR] f32, BassKernelResults)."""
    from concourse.bass_utils import run_bass_kernel_spmd

    in_maps = prep_host(**inputs, rows=ROWS, mm_dt=mm_dt)
    nc = get_program(ROWS, mm_dt)
    res = run_bass_kernel_spmd(nc, in_maps, list(range(NCORES)), trace=trace)
    out = np.concatenate(
        [unpack_out(res.results[c]["out"], ROWS) for c in range(NCORES)]
    )
    return out.reshape(B, NR).astype(np.float32), res


def kernel(**inputs):
    out, _ = run(inputs, trace=False)
    return out
